# revision 24
# baseline (speedup 1.0000x reference)
"""CODA-Prompt forward kernel for 8 TRN2 NeuronCores (data-parallel over batch).

Reference computation (forward only; stop_gradient is identity):
    K = (task_count + 1) * 10            # active pool slice, all branches
    x_mean[b,d]  = mean_n x[b,n,d]
    aq[b,k]      = (x_mean . (att[k]*nK[k])) / max(||x_mean*att[k]||, eps)
    P_[b,l,d]    = sum_k aq[b,k] * prompt[k,l,d]
    out          = concat([P_, x], axis=1)            # [B, 8+197, 768]

Device kernel per core (B=32 of 256 batches), built for DMA efficiency:
  x arrives flat+padded [B*197+1, 768].  Each batch is one fully
  sequential in-DMA in token-pair layout [99, 2, 768] (6 KB runs); the
  out-copy writes rows [205b+8, 205b+204) from the same tile plus the
  odd 197th row DRAM->DRAM straight from x, so nothing ever touches the
  P_ rows and there are no DRAM write-write hazards.  The tile's 198th
  row is the next batch's token 0 (garbage); its contribution to the
  token sum is removed with a correction DMA of rows x[b+1, 0, :].

  Token sums accumulate batch-on-partition in PSUM via indicator-
  stationary matmuls (lhsT = e_b x ones built on device from a 4 KB
  flattened-identity constant).  Batches run in 4 groups of 8: as soon
  as a group's sums are in PSUM, its stage 2 (transpose + aq) and
  stage 3 (P_ = aq @ prompt, P_ DMA) run overlapped with the remaining
  streaming, so the serial tail is only the last group's stage 2/3
  instead of all 32 batches'.

Host combines the small pool tensors:
    attnkT[d,k] = att[k,d] * nK[k,d],  attn2T[d,k] = att[k,d]^2,
    prflat[k,:] = prompt[k].reshape(6144)
aq is scale-invariant in x_mean, so the 1/197 mean scaling cancels and
the kernel works with raw token sums.
"""

import numpy as np

TOP_K = 10
LENGTH = 8
EMBED_DIM = 768
N_TOK = 197
B_FULL = 256
N_CORES = 8
B = B_FULL // N_CORES          # 32 batches per core
GB = 8                         # batches per stage-2/3 group
NG = B // GB                   # 4 groups
PF = LENGTH * EMBED_DIM        # 6144 flattened prompt row
XROWS = B * N_TOK + 1          # flat x rows incl one zero pad row
OROWS = B * (LENGTH + N_TOK)   # flat out rows
NP2 = (N_TOK + 1) // 2         # 99 token pairs per batch (last half garbage)

_PROGRAMS = {}


def _build_program(K):
    import concourse.bacc as bacc
    import concourse.mybir as mybir
    import concourse.tile as tile
    from concourse.bass import ts
    from concourse.masks import make_identity
    import concourse.bass as bass

    f32 = mybir.dt.float32
    nc = bacc.Bacc()

    x = nc.dram_tensor("x", [XROWS, EMBED_DIM], f32, kind="ExternalInput")
    prflat = nc.dram_tensor("prflat", [K, PF], f32, kind="ExternalInput")
    attnkT = nc.dram_tensor("attnkT", [EMBED_DIM, K], f32, kind="ExternalInput")
    attn2T = nc.dram_tensor("attn2T", [EMBED_DIM, K], f32, kind="ExternalInput")
    emflat = nc.dram_tensor("emflat", [1, GB * GB], f32, kind="ExternalInput")
    out = nc.dram_tensor("out", [OROWS, EMBED_DIM], f32, kind="ExternalOutput")

    with tile.TileContext(nc) as tc:
        with (
            tc.tile_pool(name="const", bufs=1) as constp,
            tc.tile_pool(name="xt", bufs=8) as xtp,
            tc.tile_pool(name="xs", bufs=6) as xsp,
            tc.tile_pool(name="grp", bufs=2) as grpp,
            tc.tile_pool(name="psg", bufs=2, space="PSUM") as psgp,
            tc.tile_pool(name="pt", bufs=1, space="PSUM") as ptp,
            tc.tile_pool(name="pnq", bufs=1, space="PSUM") as pnqp,
            tc.tile_pool(name="pp", bufs=2, space="PSUM") as ppp,
        ):
            # --- constants (gpsimd queue; big streams go on sync/scalar) ---
            ident = constp.tile([128, 128], f32)
            make_identity(nc, ident)
            prflat_sb = constp.tile([K, PF], f32)
            nc.gpsimd.dma_start(out=prflat_sb, in_=prflat[:, :])
            attnkT_sb = constp.tile([128, 6, K], f32)
            nc.gpsimd.dma_start(
                out=attnkT_sb,
                in_=attnkT[:, :].rearrange("(c p) k -> p c k", p=128))
            attn2T_sb = constp.tile([128, 6, K], f32)
            nc.gpsimd.dma_start(
                out=attn2T_sb,
                in_=attn2T[:, :].rearrange("(c p) k -> p c k", p=128))
            # correction rows: x[b+1, token 0], one tile per group of 8
            # batches (SBUF APs must start at partition 0)
            corr_g = []
            for g in range(NG):
                cg = constp.tile([GB, EMBED_DIM], f32, name=f"corr{g}")
                nc.gpsimd.dma_start(out=cg, in_=bass.AP(
                    tensor=x[:, :].tensor,
                    offset=(g * GB + 1) * N_TOK * EMBED_DIM,
                    ap=[[N_TOK * EMBED_DIM, GB], [1, EMBED_DIM]]))
                corr_g.append(cg)
            # batch-in-group indicator em[p, bi, c] = (bi == c), replicated
            # over partitions from a 4 KB host constant via a K=1 matmul
            emflat_sb = constp.tile([1, GB * GB], f32)
            nc.gpsimd.dma_start(out=emflat_sb, in_=emflat[:, :])
            onesc = constp.tile([1, 128], f32)
            nc.vector.memset(onesc, 1.0)
            em_sb = constp.tile([128, GB, GB], f32)
            pe0 = ptp.tile([128, GB * GB], f32, tag="pt", name="pt")
            nc.tensor.matmul(pe0, onesc, emflat_sb, start=True, stop=True)
            nc.vector.tensor_copy(em_sb, pe0)

            # Preheat: have PE consume each constant once so no later matmul
            # needs >1 semaphore wait.
            scr = ptp.tile([128, GB * GB], f32, tag="pt", name="pt")
            nc.tensor.matmul(scr[:1, :1], ident[:1, :1], ident[:1, :1],
                             start=True, stop=True)
            nc.tensor.matmul(scr[:1, :1], attnkT_sb[:1, 0, :1],
                             attnkT_sb[:1, 0, :1], start=True, stop=True)
            nc.tensor.matmul(scr[:1, :1], attn2T_sb[:1, 0, :1],
                             attn2T_sb[:1, 0, :1], start=True, stop=True)
            nc.tensor.matmul(scr[:1, :1], prflat_sb[:1, :1],
                             prflat_sb[:1, :1], start=True, stop=True)
            nc.tensor.matmul(scr[:1, :1], em_sb[:1, 0, :1], em_sb[:1, 0, :1],
                             start=True, stop=True)

            # Byte-balance the three DMA queues (sync/scalar HWDGE ~1.0 rel
            # rate, gpsimd SWDGE ~0.56): in-DMAs on sync with a few spilled
            # to scalar; out-DMAs on scalar with some on gpsimd.
            in_eng = [nc.sync] * B
            for b in range(4, B, 4):
                if sum(1 for e in in_eng if e is nc.scalar) < 7:
                    in_eng[b] = nc.scalar
            out_eng = [nc.scalar] * B
            for b in range(1, B, 2):
                if sum(1 for e in out_eng if e is nc.gpsimd) < 13:
                    out_eng[b] = nc.gpsimd

            def stage23(g, psh):
                """aq + P_ for batches 8g..8g+7, overlapped with streaming."""
                # garbage-row correction on the way out of PSUM
                means = grpp.tile([GB, EMBED_DIM], f32, name="means")
                for h in range(2):
                    nc.vector.tensor_sub(
                        means[:, ts(h, 384)], psh[h],
                        corr_g[g][:, ts(h, 384)])

                meansT = grpp.tile([128, 6, GB], f32, name="meansT")
                for j in range(6):
                    pt = ptp.tile([128, GB * GB], f32, tag="pt", name="pt")
                    nc.tensor.transpose(pt[:, :GB], means[:, ts(j, 128)],
                                        ident[:GB, :GB])
                    if j % 2 == 0:
                        nc.vector.tensor_copy(meansT[:, j, :], pt[:, :GB])
                    else:
                        nc.scalar.copy(meansT[:, j, :], pt[:, :GB])
                sqT = grpp.tile([128, 6, GB], f32, name="sqT")
                nc.vector.tensor_mul(sqT, meansT, meansT)

                pn = pnqp.tile([K, 2, GB], f32, name="pn")
                for j in range(6):
                    nc.tensor.matmul(pn[:, 0, :], attnkT_sb[:, j, :],
                                     meansT[:, j, :],
                                     start=(j == 0), stop=(j == 5))
                for j in range(6):
                    nc.tensor.matmul(pn[:, 1, :], attn2T_sb[:, j, :],
                                     sqT[:, j, :],
                                     start=(j == 0), stop=(j == 5))

                denom = grpp.tile([K, GB], f32, name="denom")
                nc.scalar.sqrt(denom, pn[:, 1, :])
                nc.vector.tensor_scalar_max(denom, denom, 1e-12)
                recip = grpp.tile([K, GB], f32, name="recip")
                nc.vector.reciprocal(recip, denom)
                aqT = grpp.tile([K, GB], f32, name="aqT")
                nc.vector.tensor_mul(aqT, pn[:, 0, :], recip)

                # stage 3: P_ = aq @ prflat
                p_sb = grpp.tile([GB, PF], f32, name="p_sb")
                for h in range(PF // 512):
                    pp = ppp.tile([GB, 512], f32, tag="pp", name="pp")
                    nc.tensor.matmul(pp, aqT, prflat_sb[:, ts(h, 512)],
                                     start=True, stop=True)
                    if h % 2 == 0:
                        nc.vector.tensor_copy(p_sb[:, ts(h, 512)], pp)
                    else:
                        nc.scalar.copy(p_sb[:, ts(h, 512)], pp)
                orow = (LENGTH + N_TOK) * EMBED_DIM
                nc.gpsimd.dma_start(
                    out=bass.AP(tensor=out[:, :].tensor,
                                offset=g * GB * orow,
                                ap=[[orow, GB], [1, PF]]),
                    in_=p_sb)

            # --- stage 1: stream x, copy to out rows, accumulate sums ------
            psg_tiles = {}
            for b in range(B):
                g, bi = b // GB, b % GB
                r0 = b * N_TOK
                o0 = b * (LENGTH + N_TOK) + LENGTH
                xt = xtp.tile([NP2, 2, EMBED_DIM], f32)
                in_eng[b].dma_start(
                    out=xt,
                    in_=x[r0:r0 + 2 * NP2, :].rearrange("(p u) d -> p u d",
                                                        u=2))
                # out-copy: 196 pair rows from SBUF; the odd 197th row goes
                # DRAM->DRAM from x (SBUF APs can't start at partition 98)
                out_eng[b].dma_start(
                    out=out[o0:o0 + N_TOK - 1, :].rearrange(
                        "(p u) d -> p u d", u=2),
                    in_=xt[:NP2 - 1])
                nc.gpsimd.dma_start(
                    out=out[o0 + N_TOK - 1:o0 + N_TOK, :],
                    in_=x[r0 + N_TOK - 1:r0 + N_TOK, :])
                # fold the token pairs on DVE: halves the PE streaming volume
                xs = xsp.tile([NP2, EMBED_DIM], f32)
                nc.vector.tensor_add(xs, xt[:, 0, :], xt[:, 1, :])
                if bi == 0:
                    psg_tiles[g] = [psgp.tile([GB, 384], f32, name=f"psg{h}")
                                    for h in range(2)]
                for h in range(2):
                    nc.tensor.matmul(
                        psg_tiles[g][h],
                        em_sb[:NP2, bi, :], xs[:, ts(h, 384)],
                        start=(bi == 0), stop=(bi == GB - 1))
                if bi == GB - 1:
                    stage23(g, psg_tiles.pop(g))

    nc.finalize()
    return nc


def _host_prep(prompt, attention, prompt_key, task_count):
    K = (int(task_count) + 1) * TOP_K
    pk = np.asarray(prompt_key[:K], dtype=np.float32)
    att = np.asarray(attention[:K], dtype=np.float32)
    pr = np.asarray(prompt[:K], dtype=np.float32)
    nrm = np.sqrt(np.sum(pk * pk, axis=1, keepdims=True, dtype=np.float32))
    nK = pk / np.maximum(nrm, np.float32(1e-12))
    attnkT = np.ascontiguousarray((att * nK).T)
    attn2T = np.ascontiguousarray((att * att).T)
    prflat = np.ascontiguousarray(pr.reshape(K, PF))
    return K, attnkT, attn2T, prflat


def _shard_x(x_embed, i):
    flat = x_embed[i * B:(i + 1) * B].reshape(B * N_TOK, EMBED_DIM)
    pad = np.zeros((1, EMBED_DIM), dtype=np.float32)
    return np.ascontiguousarray(np.concatenate([flat, pad], axis=0))


def _emflat():
    return np.eye(GB, dtype=np.float32).reshape(1, GB * GB)


def kernel(x_embed, prompt, attention, prompt_key, iseval, task_count,
           _want_trace=False, **_trace_kwargs):
    from concourse.bass_utils import run_bass_kernel_spmd

    x_embed = np.asarray(x_embed, dtype=np.float32)
    assert x_embed.shape == (B_FULL, N_TOK, EMBED_DIM)
    K, attnkT, attn2T, prflat = _host_prep(prompt, attention, prompt_key,
                                           task_count)

    if K not in _PROGRAMS:
        _PROGRAMS[K] = _build_program(K)
    nc = _PROGRAMS[K]

    in_maps = []
    for i in range(N_CORES):
        in_maps.append({
            "x": _shard_x(x_embed, i),
            "prflat": prflat,
            "attnkT": attnkT,
            "attn2T": attn2T,
            "emflat": _emflat(),
        })
    res = run_bass_kernel_spmd(nc, in_maps, core_ids=list(range(N_CORES)),
                               trace=_want_trace, **_trace_kwargs)
    full = np.concatenate(
        [res.results[i]["out"].reshape(
            B, LENGTH + N_TOK, EMBED_DIM) for i in range(N_CORES)],
        axis=0)
    if _want_trace:
        return full, res
    return full


# revision 27
# speedup vs baseline: 1.2986x; 1.2986x over previous
"""CODA-Prompt forward kernel for 8 TRN2 NeuronCores (data-parallel over batch).

Reference computation (forward only; stop_gradient is identity):
    K = (task_count + 1) * 10            # active pool slice, all branches
    x_mean[b,d]  = mean_n x[b,n,d]
    aq[b,k]      = (x_mean . (att[k]*nK[k])) / max(||x_mean*att[k]||, eps)
    P_[b,l,d]    = sum_k aq[b,k] * prompt[k,l,d]
    out          = concat([P_, x], axis=1)            # [B, 8+197, 768]

Device kernel per core (B=32 of 256 batches), built for DMA efficiency:
  x arrives flat+padded [B*197+1, 768].  Each batch is one fully
  sequential in-DMA in token-pair layout [99, 2, 768] (6 KB runs); the
  out-copy writes rows [205b+8, 205b+204) from the same tile plus the
  odd 197th row DRAM->DRAM straight from x, so nothing ever touches the
  P_ rows and there are no DRAM write-write hazards.  The tile's 198th
  row is the next batch's token 0 (garbage); its contribution to the
  token sum is removed with a correction DMA of rows x[b+1, 0, :].

  Token sums accumulate batch-on-partition in PSUM via indicator-
  stationary matmuls (lhsT = e_b x ones built on device from a 4 KB
  flattened-identity constant).  Batches run in 4 groups of 8: as soon
  as a group's sums are in PSUM, its stage 2 (transpose + aq) and
  stage 3 (P_ = aq @ prompt, P_ DMA) run overlapped with the remaining
  streaming, so the serial tail is only the last group's stage 2/3
  instead of all 32 batches'.

Host combines the small pool tensors:
    attnkT[d,k] = att[k,d] * nK[k,d],  attn2T[d,k] = att[k,d]^2,
    prflat[k,:] = prompt[k].reshape(6144)
aq is scale-invariant in x_mean, so the 1/197 mean scaling cancels and
the kernel works with raw token sums.
"""

import numpy as np

TOP_K = 10
LENGTH = 8
EMBED_DIM = 768
N_TOK = 197
B_FULL = 256
N_CORES = 8
B = B_FULL // N_CORES          # 32 batches per core
GB = 8                         # batches per stage-2/3 group
NG = B // GB                   # 4 groups
PF = LENGTH * EMBED_DIM        # 6144 flattened prompt row
XROWS = B * N_TOK + 1          # flat x rows incl one zero pad row
OROWS = B * (LENGTH + N_TOK)   # flat out rows
NP2 = (N_TOK + 1) // 2         # 99 token pairs per batch (last half garbage)

_PROGRAMS = {}


def _build_program(K):
    import concourse.bacc as bacc
    import concourse.mybir as mybir
    import concourse.tile as tile
    from concourse.bass import ts
    from concourse.masks import make_identity
    import concourse.bass as bass

    f32 = mybir.dt.float32
    nc = bacc.Bacc()

    x = nc.dram_tensor("x", [XROWS, EMBED_DIM], f32, kind="ExternalInput")
    prflat = nc.dram_tensor("prflat", [K, PF], f32, kind="ExternalInput")
    attnkT = nc.dram_tensor("attnkT", [EMBED_DIM, K], f32, kind="ExternalInput")
    attn2T = nc.dram_tensor("attn2T", [EMBED_DIM, K], f32, kind="ExternalInput")
    emflat = nc.dram_tensor("emflat", [1, GB * GB], f32, kind="ExternalInput")
    out = nc.dram_tensor("out", [OROWS, EMBED_DIM], f32, kind="ExternalOutput")

    with tile.TileContext(nc) as tc:
        with (
            tc.tile_pool(name="const", bufs=1) as constp,
            tc.tile_pool(name="xt", bufs=8) as xtp,
            tc.tile_pool(name="xs", bufs=6) as xsp,
            tc.tile_pool(name="grp", bufs=2) as grpp,
            tc.tile_pool(name="psg", bufs=2, space="PSUM") as psgp,
            tc.tile_pool(name="pt", bufs=1, space="PSUM") as ptp,
            tc.tile_pool(name="pnq", bufs=1, space="PSUM") as pnqp,
            tc.tile_pool(name="pp", bufs=2, space="PSUM") as ppp,
        ):
            # --- constants (gpsimd queue; big streams go on sync/scalar) ---
            ident = constp.tile([128, 128], f32)
            make_identity(nc, ident)
            prflat_sb = constp.tile([K, PF], f32)
            nc.gpsimd.dma_start(out=prflat_sb, in_=prflat[:, :])
            attnkT_sb = constp.tile([128, 6, K], f32)
            nc.gpsimd.dma_start(
                out=attnkT_sb,
                in_=attnkT[:, :].rearrange("(c p) k -> p c k", p=128))
            attn2T_sb = constp.tile([128, 6, K], f32)
            nc.gpsimd.dma_start(
                out=attn2T_sb,
                in_=attn2T[:, :].rearrange("(c p) k -> p c k", p=128))
            # correction rows: x[b+1, token 0], one tile per group of 8
            # batches (SBUF APs must start at partition 0)
            corr_g = []
            for g in range(NG):
                cg = constp.tile([GB, EMBED_DIM], f32, name=f"corr{g}")
                nc.gpsimd.dma_start(out=cg, in_=bass.AP(
                    tensor=x[:, :].tensor,
                    offset=(g * GB + 1) * N_TOK * EMBED_DIM,
                    ap=[[N_TOK * EMBED_DIM, GB], [1, EMBED_DIM]]))
                corr_g.append(cg)
            # batch-in-group indicator em[p, bi, c] = (bi == c), replicated
            # over partitions from a 4 KB host constant via a K=1 matmul
            emflat_sb = constp.tile([1, GB * GB], f32)
            nc.gpsimd.dma_start(out=emflat_sb, in_=emflat[:, :])
            onesc = constp.tile([1, 128], f32)
            nc.vector.memset(onesc, 1.0)
            em_sb = constp.tile([128, GB, GB], f32)
            pe0 = ptp.tile([128, GB * GB], f32, tag="pt", name="pt")
            nc.tensor.matmul(pe0, onesc, emflat_sb, start=True, stop=True)
            nc.vector.tensor_copy(em_sb, pe0)

            # Preheat: have PE consume each constant once so no later matmul
            # needs >1 semaphore wait.
            scr = ptp.tile([128, GB * GB], f32, tag="pt", name="pt")
            nc.tensor.matmul(scr[:1, :1], ident[:1, :1], ident[:1, :1],
                             start=True, stop=True)
            nc.tensor.matmul(scr[:1, :1], attnkT_sb[:1, 0, :1],
                             attnkT_sb[:1, 0, :1], start=True, stop=True)
            nc.tensor.matmul(scr[:1, :1], attn2T_sb[:1, 0, :1],
                             attn2T_sb[:1, 0, :1], start=True, stop=True)
            nc.tensor.matmul(scr[:1, :1], prflat_sb[:1, :1],
                             prflat_sb[:1, :1], start=True, stop=True)
            nc.tensor.matmul(scr[:1, :1], em_sb[:1, 0, :1], em_sb[:1, 0, :1],
                             start=True, stop=True)

            # Byte-balance the three DMA queues (sync/scalar HWDGE ~1.0 rel
            # rate, gpsimd SWDGE ~0.56): in-DMAs on sync with a few spilled
            # to scalar; out-DMAs on scalar with some on gpsimd.
            in_eng = [nc.sync] * B
            for b in range(4, B, 4):
                if sum(1 for e in in_eng if e is nc.scalar) < 7:
                    in_eng[b] = nc.scalar
            out_eng = [nc.scalar] * B
            for b in range(1, B, 2):
                if sum(1 for e in out_eng if e is nc.gpsimd) < 13:
                    out_eng[b] = nc.gpsimd

            # aq columns for all batches, filled per group as sums complete
            aqT_all = constp.tile([K, B], f32)

            def stage2(g, psh):
                """aq for batches 8g..8g+7, overlapped with streaming."""
                # garbage-row correction on the way out of PSUM
                means = grpp.tile([GB, EMBED_DIM], f32, name="means")
                for h in range(2):
                    nc.vector.tensor_sub(
                        means[:, ts(h, 384)], psh[h],
                        corr_g[g][:, ts(h, 384)])

                meansT = grpp.tile([128, 6, GB], f32, name="meansT")
                for j in range(6):
                    pt = ptp.tile([128, GB * GB], f32, tag="pt", name="pt")
                    nc.tensor.transpose(pt[:, :GB], means[:, ts(j, 128)],
                                        ident[:GB, :GB])
                    if j % 2 == 0:
                        nc.vector.tensor_copy(meansT[:, j, :], pt[:, :GB])
                    else:
                        nc.scalar.copy(meansT[:, j, :], pt[:, :GB])
                sqT = grpp.tile([128, 6, GB], f32, name="sqT")
                nc.vector.tensor_mul(sqT, meansT, meansT)

                pn = pnqp.tile([K, 2, GB], f32, name="pn")
                for j in range(6):
                    nc.tensor.matmul(pn[:, 0, :], attnkT_sb[:, j, :],
                                     meansT[:, j, :],
                                     start=(j == 0), stop=(j == 5))
                for j in range(6):
                    nc.tensor.matmul(pn[:, 1, :], attn2T_sb[:, j, :],
                                     sqT[:, j, :],
                                     start=(j == 0), stop=(j == 5))

                denom = grpp.tile([K, GB], f32, name="denom")
                nc.scalar.sqrt(denom, pn[:, 1, :])
                nc.vector.tensor_scalar_max(denom, denom, 1e-12)
                recip = grpp.tile([K, GB], f32, name="recip")
                nc.vector.reciprocal(recip, denom)
                nc.vector.tensor_mul(aqT_all[:, g * GB:(g + 1) * GB],
                                     pn[:, 0, :], recip)

            # every batch's odd 197th row in one strided DRAM->DRAM DMA
            # (SBUF APs can't start at partition 98); rows are untouched by
            # anything else, so it can fire immediately.
            orow = (LENGTH + N_TOK) * EMBED_DIM
            nc.gpsimd.dma_start(
                out=bass.AP(tensor=out[:, :].tensor,
                            offset=(LENGTH + N_TOK - 1) * EMBED_DIM,
                            ap=[[orow, B], [1, EMBED_DIM]]),
                in_=bass.AP(tensor=x[:, :].tensor,
                            offset=(N_TOK - 1) * EMBED_DIM,
                            ap=[[N_TOK * EMBED_DIM, B], [1, EMBED_DIM]]))

            # --- stage 1: stream x, copy to out rows, accumulate sums ------
            psg_tiles = {}
            for b in range(B):
                g, bi = b // GB, b % GB
                r0 = b * N_TOK
                o0 = b * (LENGTH + N_TOK) + LENGTH
                xt = xtp.tile([NP2, 2, EMBED_DIM], f32)
                in_eng[b].dma_start(
                    out=xt,
                    in_=x[r0:r0 + 2 * NP2, :].rearrange("(p u) d -> p u d",
                                                        u=2))
                # out-copy: the 196 pair rows (197th went DRAM->DRAM above)
                out_eng[b].dma_start(
                    out=out[o0:o0 + N_TOK - 1, :].rearrange(
                        "(p u) d -> p u d", u=2),
                    in_=xt[:NP2 - 1])
                # fold the token pairs on DVE: halves the PE streaming volume
                xs = xsp.tile([NP2, EMBED_DIM], f32)
                nc.vector.tensor_add(xs, xt[:, 0, :], xt[:, 1, :])
                if bi == 0:
                    psg_tiles[g] = [psgp.tile([GB, 384], f32, name=f"psg{h}")
                                    for h in range(2)]
                for h in range(2):
                    nc.tensor.matmul(
                        psg_tiles[g][h],
                        em_sb[:NP2, bi, :], xs[:, ts(h, 384)],
                        start=(bi == 0), stop=(bi == GB - 1))
                if bi == GB - 1:
                    stage2(g, psg_tiles.pop(g))

            # --- stage 3 (single shot, M=32): P_ = aq @ prflat -------------
            p_sb = constp.tile([B, PF], f32)
            for h in range(PF // 512):
                pp = ppp.tile([B, 512], f32, tag="pp", name="pp")
                nc.tensor.matmul(pp, aqT_all, prflat_sb[:, ts(h, 512)],
                                 start=True, stop=True)
                if h % 2 == 0:
                    nc.vector.tensor_copy(p_sb[:, ts(h, 512)], pp)
                else:
                    nc.scalar.copy(p_sb[:, ts(h, 512)], pp)
            nc.scalar.dma_start(
                out=bass.AP(tensor=out[:, :].tensor, offset=0,
                            ap=[[orow, B], [1, PF]]),
                in_=p_sb)

    nc.finalize()
    return nc


def _host_prep(prompt, attention, prompt_key, task_count):
    K = (int(task_count) + 1) * TOP_K
    pk = np.asarray(prompt_key[:K], dtype=np.float32)
    att = np.asarray(attention[:K], dtype=np.float32)
    pr = np.asarray(prompt[:K], dtype=np.float32)
    nrm = np.sqrt(np.sum(pk * pk, axis=1, keepdims=True, dtype=np.float32))
    nK = pk / np.maximum(nrm, np.float32(1e-12))
    attnkT = np.ascontiguousarray((att * nK).T)
    attn2T = np.ascontiguousarray((att * att).T)
    prflat = np.ascontiguousarray(pr.reshape(K, PF))
    return K, attnkT, attn2T, prflat


def _shard_x(x_embed, i):
    flat = x_embed[i * B:(i + 1) * B].reshape(B * N_TOK, EMBED_DIM)
    pad = np.zeros((1, EMBED_DIM), dtype=np.float32)
    return np.ascontiguousarray(np.concatenate([flat, pad], axis=0))


def _emflat():
    return np.eye(GB, dtype=np.float32).reshape(1, GB * GB)


def kernel(x_embed, prompt, attention, prompt_key, iseval, task_count,
           _want_trace=False, **_trace_kwargs):
    from concourse.bass_utils import run_bass_kernel_spmd

    x_embed = np.asarray(x_embed, dtype=np.float32)
    assert x_embed.shape == (B_FULL, N_TOK, EMBED_DIM)
    K, attnkT, attn2T, prflat = _host_prep(prompt, attention, prompt_key,
                                           task_count)

    if K not in _PROGRAMS:
        _PROGRAMS[K] = _build_program(K)
    nc = _PROGRAMS[K]

    in_maps = []
    for i in range(N_CORES):
        in_maps.append({
            "x": _shard_x(x_embed, i),
            "prflat": prflat,
            "attnkT": attnkT,
            "attn2T": attn2T,
            "emflat": _emflat(),
        })
    res = run_bass_kernel_spmd(nc, in_maps, core_ids=list(range(N_CORES)),
                               trace=_want_trace, **_trace_kwargs)
    full = np.concatenate(
        [res.results[i]["out"].reshape(
            B, LENGTH + N_TOK, EMBED_DIM) for i in range(N_CORES)],
        axis=0)
    if _want_trace:
        return full, res
    return full
